# revision 1
# baseline (speedup 1.0000x reference)
"""Trainium2 Bass kernel for multi-head self-attention (dense transformer block).

Problem: x[4, 2048, 1024], w_qkv[3072, 1024], w_out[1024, 1024], b_out[1024]
  qkv = x @ w_qkv.T, rearranged 'b t (d k h) -> k b h t d' (k=3, h=16)
  attn = softmax(q @ k.T * DIM**-0.5); out = (attn @ v) concat heads @ w_out.T + b_out

Sharding (8 cores): data-parallel over batch b (4) x tensor-parallel over
head-groups (2 groups of 8 heads).  Each core gets x[b] (pre-transposed on
host), the w_qkv rows for its 8 heads (pre-gathered/transposed on host), and
the matching w_out columns; it produces a partial [T, DIM] output which the
host sums per batch pair (the "all-reduce" of the row-sharded w_out matmul)
and adds b_out.

Device-side dataflow per core:
  phase 1: qT, kT = (w.T @ xT) in [head*dh, t] layout (bf16), V in [t, head*dh]
           layout with a ones-column appended per head (bf16).
  phase 2: per (i-chunk of 512 queries, head): S^T[j, i] = kT.T @ qT (bf16
           matmuls, two heads packed in the 128-partition dim via PE row
           tiling), exp on ScalarE with the softmax scale folded in, then
           out[i, 65] = expST.T @ [v | 1] accumulating over j, giving the
           unnormalized attention output and its softmax denominator in one
           matmul; normalize with a per-partition reciprocal multiply.
  phase 3: PE-transpose attn [t, o] -> [o, t], project with w_out.T (fp32r),
           DMA partial outputs.
"""

import math
from contextlib import ExitStack
from dataclasses import dataclass

import numpy as np

import concourse.bass as bass
import concourse.mybir as mybir
import concourse.tile as tile
from concourse import bacc
from concourse.bass_utils import run_bass_kernel_spmd
from concourse.masks import make_identity

F32 = mybir.dt.float32
F32R = mybir.dt.float32r
BF16 = mybir.dt.bfloat16
P = 128


@dataclass(frozen=True)
class Cfg:
    T: int = 2048      # sequence length
    DIM: int = 1024    # model dim (= qkv contraction dim)
    NH: int = 8        # heads per core
    DH: int = 64       # head dim
    SCALE: float = 1024.0 ** -0.5

    @property
    def CB(self):      # contraction blocks of 128 over DIM
        return self.DIM // P

    @property
    def OD(self):      # per-core attention width = NH*DH
        return self.NH * self.DH

    @property
    def OB(self):      # o-blocks of 128 (= head pairs, 2 x 64)
        return self.OD // P

    @property
    def JB(self):      # key blocks of 128
        return self.T // P

    @property
    def ICSZ(self):    # query chunk size
        return min(256, self.T)

    @property
    def NIC(self):     # number of query chunks
        return self.T // self.ICSZ

    @property
    def IB(self):      # query blocks of 128 per chunk
        return self.ICSZ // P

    @property
    def TCH(self):     # t-chunk for phase-1 rhs streaming
        return min(256, self.T)

    @property
    def NTCH(self):
        return self.T // self.TCH

    @property
    def OCC(self):     # output-column chunk for the final projection
        return min(512, self.DIM)

    @property
    def NOCC(self):
        return self.DIM // self.OCC


def _emit_kernel(tc, cfg, xT, wq, wk, wv, woT, out):
    """Emit the per-core attention kernel under an open TileContext."""
    nc = tc.nc
    c = cfg
    VW = c.DH + 1  # per-head V width incl. ones column

    ctx = ExitStack()
    with ctx:
        persist = ctx.enter_context(tc.tile_pool(name="persist", bufs=1))
        mmp = ctx.enter_context(tc.tile_pool(name="mmp", bufs=2, space="PSUM"))
        smp = ctx.enter_context(tc.tile_pool(name="smp", bufs=4, space="PSUM"))

        qT_sb = persist.tile([P, c.OB, c.T], BF16, name="qT_sb", tag="qT")
        kT_sb = persist.tile([P, c.OB, c.T], BF16, name="kT_sb", tag="kT")
        v_sb = persist.tile([P, c.JB, c.NH, VW], BF16, name="v_sb", tag="v")
        woT_sb = persist.tile([P, c.OB, c.DIM], F32R, name="woT_sb", tag="woT")
        ident = persist.tile([P, P], F32, name="ident", tag="ident")

        make_identity(nc, ident)
        nc.gpsimd.memset(v_sb[:, :, :, c.DH : c.DH + 1], 1.0)

        xT_r = xT.rearrange("(cb p) t -> p cb t", p=P)
        wq_r = wq.rearrange("(cb p) o -> p cb o", p=P)
        wk_r = wk.rearrange("(cb p) o -> p cb o", p=P)
        wv_r = wv.rearrange("(cb p) o -> p cb o", p=P)

        # ------- phase 1: V and kT (qT is computed just-in-time in phase 2,
        # so ScalarE's exp work starts as soon as kT + the first qT chunk
        # exist) -------
        with (
            tc.tile_pool(name="xp", bufs=16) as xp,
            tc.tile_pool(name="wp", bufs=1) as wp,
        ):
            # per-cb wk and x tiles: Tile's dependency tracking is per-tile,
            # so cb-granular tiles let the first K matmul start after ~0.5MB
            # of DMA instead of waiting for the full 4MB wk+x load.
            wv_sb = wp.tile([P, c.CB, c.OD], F32R, name="wv_sb", tag="wv")
            wk_cb = []
            for cb in range(c.CB):
                wk_t = wp.tile([P, c.OD], F32R, name=f"wk_{cb}", tag=f"wk{cb}")
                nc.sync.dma_start(out=wk_t, in_=wk_r[:, cb, :].bitcast(F32R))
                wk_cb.append(wk_t)

            for tch in range(c.NTCH):
                tsl = bass.ts(tch, c.TCH)
                x_cb = []
                for cb in range(c.CB):
                    x_t = xp.tile([P, c.TCH], F32R, name=f"x_sb{cb}", tag="x")
                    nc.sync.dma_start(
                        out=x_t, in_=xT_r[:, cb, tsl].bitcast(F32R)
                    )
                    x_cb.append(x_t)
                if tch == 0:
                    # wv after x0: the first K matmuls need wk+x0, not wv
                    nc.sync.dma_start(out=wv_sb, in_=wv_r.bitcast(F32R))
                    nc.sync.dma_start(
                        out=woT_sb,
                        in_=woT.rearrange("(ob p) n -> p ob n", p=P).bitcast(F32R),
                    )

                # kT first: the first scores unit waits on the LAST kT chunk,
                # so finish all of kT as early as possible.
                for ob in range(c.OB):
                    ps = mmp.tile([P, c.TCH], F32, name="ps_qk", tag="mm")
                    for cb in range(c.CB):
                        nc.tensor.matmul(
                            ps,
                            wk_cb[cb][:, bass.ts(ob, P)],
                            x_cb[cb],
                            start=(cb == 0),
                            stop=(cb == c.CB - 1),
                        )
                    nc.vector.tensor_copy(out=kT_sb[:, ob, tsl], in_=ps)

                # V: psum[t-block 128, OD] += xT[cb, tb].T @ wv[cb, :]
                ps_v = [
                    smp.tile([P, c.OD], F32, name=f"ps_v{i}", tag="sm")
                    for i in range(c.TCH // P)
                ]
                for cb in range(c.CB):
                    for tbl in range(c.TCH // P):
                        nc.tensor.matmul(
                            ps_v[tbl],
                            x_cb[cb][:, bass.ts(tbl, P)],
                            wv_sb[:, cb, :],
                            start=(cb == 0),
                            stop=(cb == c.CB - 1),
                        )
                for tbl in range(c.TCH // P):
                    tb = tch * (c.TCH // P) + tbl
                    nc.vector.tensor_copy(
                        out=v_sb[:, tb, :, 0 : c.DH],
                        in_=ps_v[tbl].rearrange("p (h d) -> p h d", h=c.NH),
                    )

        # ---------------- phase 2/3: attention + projection ----------------
        with (
            tc.tile_pool(name="ep", bufs=4) as ep,
            tc.tile_pool(name="ap", bufs=2) as ap,
            tc.tile_pool(name="atp", bufs=2) as atp,
            tc.tile_pool(name="op", bufs=2) as op,
            tc.tile_pool(name="rp", bufs=2) as rp,
            tc.tile_pool(name="xq", bufs=1) as xq,
            tc.tile_pool(name="wqp", bufs=1) as wqp,
        ):
            attn_tiles = {}
            attnT_tiles = {}
            wq_sb = wqp.tile([P, c.CB, c.OD], F32R, name="wq_sb", tag="wq")
            nc.sync.dma_start(out=wq_sb, in_=wq_r.bitcast(F32R))

            def emit_q_dma(tch):
                """Start the x DMA for a just-in-time qT chunk."""
                tsl = bass.ts(tch, c.TCH)
                x2_sb = xq.tile([P, c.CB, c.TCH], F32R, name="x2_sb", tag="xq")
                nc.sync.dma_start(out=x2_sb, in_=xT_r[:, :, tsl].bitcast(F32R))
                return x2_sb

            def emit_q_ob(x2_sb, tch, ob):
                """One o-block piece of the JIT qT projection."""
                ps = smp.tile([P, c.TCH], F32, name="ps_q2", tag="sm")
                for cb in range(c.CB):
                    nc.tensor.matmul(
                        ps,
                        wq_sb[:, cb, bass.ts(ob, P)],
                        x2_sb[:, cb, :],
                        start=(cb == 0),
                        stop=(cb == c.CB - 1),
                    )
                nc.vector.tensor_copy(
                    out=qT_sb[:, ob, bass.ts(tch, c.TCH)], in_=ps
                )

            def emit_q(tch):
                x2_sb = emit_q_dma(tch)
                for ob in range(c.OB):
                    emit_q_ob(x2_sb, tch, ob)

            def emit_scores(ic, hp):
                """S^T then exp for head pair hp at query chunk ic."""
                isl = bass.ts(ic, c.ICSZ)
                e_pair = []
                for half in range(2):
                    e_pair.append(
                        ep.tile([P, c.JB, c.ICSZ], BF16,
                                name=f"e_{ic}_{hp}_{half}", tag="e")
                    )
                # group JJ key-blocks per PSUM tile so each exp activation
                # covers FD = JJ*ICSZ = 1024 elements (amortizes ACT overhead)
                JJ = min(4096 // (4 * c.ICSZ), c.JB)
                for g in range(c.JB // JJ):
                    ps = []
                    for half in range(2):
                        ps.append(
                            mmp.tile([P, JJ, c.ICSZ], F32,
                                     name=f"ps_s{half}", tag="mm")
                        )
                    for jj in range(JJ):
                        jb = g * JJ + jj
                        for half in range(2):
                            rows = slice(64 * half, 64 * half + 64)
                            nc.tensor.matmul(
                                ps[half][:, jj, :],
                                kT_sb[rows, hp, bass.ts(jb, P)],
                                qT_sb[rows, hp, isl],
                                start=True,
                                stop=True,
                            )
                    for half in range(2):
                        nc.scalar.activation(
                            out=e_pair[half][:, g * JJ : (g + 1) * JJ, :],
                            in_=ps[half],
                            func=mybir.ActivationFunctionType.Exp,
                            scale=c.SCALE,
                        )
                return e_pair

            def emit_av(ic, hp, e_pair):
                """attn[i, dh] = norm(expST.T @ [v|1]) for both heads of hp,
                then PE-transpose this head pair's 128 columns while hot."""
                if ic not in attn_tiles:
                    attn_tiles[ic] = ap.tile(
                        [P, c.IB, c.OD], F32, name=f"attn_{ic}", tag="attn"
                    )
                attn_sb = attn_tiles[ic]
                for half in range(2):
                    h = 2 * hp + half
                    e = e_pair[half]
                    for ib in range(c.IB):
                        ps_av = smp.tile([P, c.OD], F32, name="ps_av", tag="sm")
                        for jb in range(c.JB):
                            nc.tensor.matmul(
                                ps_av[:, 0:VW],
                                e[:, jb, bass.ts(ib, P)],
                                v_sb[:, jb, h, :],
                                start=(jb == 0),
                                stop=(jb == c.JB - 1),
                            )
                        rec = rp.tile([P, 1], F32, name="rec", tag="rec")
                        nc.vector.reciprocal(rec, ps_av[:, c.DH : c.DH + 1])
                        nc.vector.tensor_scalar_mul(
                            out=attn_sb[:, ib, bass.ts(h, c.DH)],
                            in0=ps_av[:, 0 : c.DH],
                            scalar1=rec,
                        )
                if ic not in attnT_tiles:
                    attnT_tiles[ic] = atp.tile(
                        [P, c.OB, c.ICSZ], F32R, name=f"attnT_{ic}", tag="attnT"
                    )
                attnT_sb = attnT_tiles[ic]
                for ib in range(c.IB):
                    ps_tp = smp.tile([P, P], F32, name="ps_tp", tag="sm")
                    nc.tensor.transpose(
                        ps_tp, attn_sb[:, ib, bass.ts(hp, P)], ident
                    )
                    nc.vector.tensor_copy(
                        out=attnT_sb[:, hp, bass.ts(ib, P)], in_=ps_tp
                    )
                if hp == c.OB - 1:
                    attn_tiles.pop(ic)

            def emit_finalize(ic, half):
                """One half of the output projection for a finished chunk,
                split so the PE burst stays smaller than ScalarE's slack."""
                attnT_sb = attnT_tiles[ic]
                h_tb = c.IB // 2
                for tb in range(half * h_tb, (half + 1) * h_tb):
                    for occ in range(c.NOCC):
                        ps_o = smp.tile([P, c.OCC], F32, name="ps_o", tag="sm")
                        for ob in range(c.OB):
                            nc.tensor.matmul(
                                ps_o,
                                attnT_sb[:, ob, bass.ts(tb, P)],
                                woT_sb[:, ob, bass.ts(occ, c.OCC)],
                                start=(ob == 0),
                                stop=(ob == c.OB - 1),
                            )
                        o_sb = op.tile([P, c.OCC], F32, name="o_sb", tag="ost")
                        nc.vector.tensor_copy(out=o_sb, in_=ps_o)
                        t0 = ic * c.ICSZ + tb * P
                        nc.sync.dma_start(
                            out=out[t0 : t0 + P, bass.ts(occ, c.OCC)],
                            in_=o_sb,
                        )
                if half == 1:
                    attnT_tiles.pop(ic)

            # software pipeline: scores(n) ... av(n-1) so the PE never
            # head-of-line blocks on ScalarE's exp of the current unit.  The
            # V projection is emitted after the first scores unit: scores/exp
            # don't need V, so ScalarE starts ~40us earlier and chews the
            # first unit's exps while the PE projects V.
            assert c.ICSZ == c.TCH and c.NIC == c.NTCH
            units = [(ic, hp) for ic in range(c.NIC) for hp in range(c.OB)]
            prev = None
            x2_next = None
            fin_queue = []
            for ic, hp in units:
                if ic == 0 and hp == 0:
                    emit_q(0)
                e_pair = emit_scores(ic, hp)
                if prev is not None:
                    emit_av(*prev)
                    if prev[1] == c.OB - 1:
                        fin_queue.extend([(prev[0], 0), (prev[0], 1)])
                if fin_queue:
                    emit_finalize(*fin_queue.pop(0))
                # spread the next chunk's JIT qT projection one o-block per
                # unit, so it never opens a bubble in ScalarE's exp stream
                if ic + 1 < c.NIC:
                    if hp == 0:
                        x2_next = emit_q_dma(ic + 1)
                    emit_q_ob(x2_next, ic + 1, hp)
                prev = (ic, hp, e_pair)
            emit_av(*prev)
            fin_queue.extend([(prev[0], 0), (prev[0], 1)])
            for f in fin_queue:
                emit_finalize(*f)


def build_nc(cfg: Cfg = Cfg(), reps: int = 1):
    nc = bacc.Bacc()
    xT = nc.declare_dram_parameter("xT", [cfg.DIM, cfg.T], F32, isOutput=False)
    wq = nc.declare_dram_parameter("wq", [cfg.DIM, cfg.OD], F32, isOutput=False)
    wk = nc.declare_dram_parameter("wk", [cfg.DIM, cfg.OD], F32, isOutput=False)
    wv = nc.declare_dram_parameter("wv", [cfg.DIM, cfg.OD], F32, isOutput=False)
    woT = nc.declare_dram_parameter("woT", [cfg.OD, cfg.DIM], F32, isOutput=False)
    out = nc.declare_dram_parameter("out", [cfg.T, cfg.DIM], F32, isOutput=True)
    with tile.TileContext(nc) as tc:
        for _ in range(reps):
            _emit_kernel(tc, cfg, xT[:], wq[:], wk[:], wv[:], woT[:], out[:])
    nc.finalize()
    return nc


def prepare_core_inputs(x, w_qkv, w_out, b, g, cfg: Cfg, n_groups: int):
    """Host-side shard prep for core (batch b, head-group g)."""
    H = cfg.NH * n_groups
    d = np.arange(cfg.DH)
    heads = np.arange(cfg.NH * g, cfg.NH * (g + 1))
    # w_qkv row for (k, head h, dim d) is d*(3*H) + k*H + h
    def gather(k_idx):
        rows = (d[None, :] * (3 * H) + k_idx * H + heads[:, None]).reshape(-1)
        return np.ascontiguousarray(w_qkv[rows, :].T, dtype=np.float32)

    return {
        "xT": np.ascontiguousarray(x[b].T, dtype=np.float32),
        "wq": gather(0),
        "wk": gather(1),
        "wv": gather(2),
        "woT": np.ascontiguousarray(
            w_out[:, cfg.OD * g : cfg.OD * (g + 1)].T, dtype=np.float32
        ),
    }


_NC_CACHE = {}


def _get_nc(cfg: Cfg):
    if cfg not in _NC_CACHE:
        _NC_CACHE[cfg] = build_nc(cfg)
    return _NC_CACHE[cfg]


def run(x, w_qkv, w_out, b_out, trace=False):
    """Shard, execute on 8 cores, gather. Returns (out, BassKernelResults)."""
    cfg = Cfg()
    B, T, DIM = x.shape
    assert (T, DIM) == (cfg.T, cfg.DIM), (x.shape, cfg)
    n_groups = 2
    nc = _get_nc(cfg)
    in_maps = [
        prepare_core_inputs(x, w_qkv, w_out, b, g, cfg, n_groups)
        for b in range(B)
        for g in range(n_groups)
    ]
    res = run_bass_kernel_spmd(
        nc, in_maps, core_ids=list(range(len(in_maps))), trace=trace
    )
    out = np.empty((B, T, DIM), dtype=np.float32)
    for b in range(B):
        out[b] = res.results[2 * b]["out"] + res.results[2 * b + 1]["out"]
    out += b_out.astype(np.float32)
    return out, res


def _make_pjrt_fn(nc, in_maps):
    """Build a non-donating jitted 8-core runner for a prebuilt nc."""
    import jax
    import numpy as np_
    from jax.sharding import Mesh, PartitionSpec
    from jax.experimental.shard_map import shard_map

    from concourse import bass2jax

    bass2jax.install_neuronx_cc_hook()
    n_cores = len(in_maps)
    partition_name = nc.partition_id_tensor.name if nc.partition_id_tensor else None
    in_names, out_names, out_avals, zero_outs = [], [], [], []
    for alloc in nc.m.functions[0].allocations:
        if not isinstance(alloc, mybir.MemoryLocationSet):
            continue
        name = alloc.memorylocations[0].name
        if alloc.kind == "ExternalInput":
            if name != partition_name:
                in_names.append(name)
        elif alloc.kind == "ExternalOutput":
            shape = tuple(alloc.tensor_shape)
            dtype = mybir.dt.np(alloc.dtype)
            out_names.append(name)
            out_avals.append(jax.core.ShapedArray(shape, dtype))
            zero_outs.append(np_.zeros(shape, dtype))
    n_params = len(in_names)
    all_in_names = in_names + out_names
    if partition_name is not None:
        all_in_names = all_in_names + [partition_name]

    def _body(*args):
        operands = list(args)
        if partition_name is not None:
            operands.append(bass2jax.partition_id_tensor())
        return tuple(
            bass2jax._bass_exec_p.bind(
                *operands,
                out_avals=tuple(out_avals),
                in_names=tuple(all_in_names),
                out_names=tuple(out_names),
                lowering_input_output_aliases=(),
                sim_require_finite=True,
                sim_require_nnan=True,
                nc=nc,
            )
        )

    devices = jax.devices()[:n_cores]
    mesh = Mesh(np_.asarray(devices), ("core",))
    nin = n_params + len(out_names)
    f = jax.jit(
        shard_map(
            _body,
            mesh=mesh,
            in_specs=(PartitionSpec("core"),) * nin,
            out_specs=(PartitionSpec("core"),) * len(out_names),
            check_rep=False,
        ),
        keep_unused=True,
    )
    concat_in = [
        np_.concatenate([np_.asarray(in_maps[c][n]) for c in range(n_cores)], axis=0)
        for n in in_names
    ] + [np_.zeros((n_cores * z.shape[0], *z.shape[1:]), z.dtype) for z in zero_outs]
    dev_in = jax.device_put(concat_in)
    return f, dev_in


def _time_fn(f, dev_in, calls=4, rounds=6):
    import time

    import jax

    r = f(*dev_in)
    jax.block_until_ready(r)
    best = float("inf")
    for _ in range(rounds):
        t0 = time.perf_counter()
        rs = [f(*dev_in) for _ in range(calls)]
        jax.block_until_ready(rs)
        best = min(best, (time.perf_counter() - t0) / calls)
    return best


def time_hw(x, w_qkv, w_out, b_out, reps=(4, 36)):
    """Marginal-cost HW timing: per-call time of an R2-repeat NEFF minus an
    R1-repeat NEFF, over (R2-R1), cancels the axon dispatch overhead."""
    cfg = Cfg()
    B = x.shape[0]
    in_maps = [
        prepare_core_inputs(x, w_qkv, w_out, b, g, cfg, 2)
        for b in range(B)
        for g in range(2)
    ]
    r1, r2 = reps
    ncA = build_nc(cfg, reps=r1)
    fA, devA = _make_pjrt_fn(ncA, in_maps)
    tA = _time_fn(fA, devA)
    ncB = build_nc(cfg, reps=r2)
    fB, devB = _make_pjrt_fn(ncB, in_maps)
    tB = _time_fn(fB, devB)
    per_exec = (tB - tA) / (r2 - r1)
    return tA, per_exec


def kernel(x, w_qkv, w_out, b_out):
    x = np.asarray(x, dtype=np.float32)
    w_qkv = np.asarray(w_qkv, dtype=np.float32)
    w_out = np.asarray(w_out, dtype=np.float32)
    b_out = np.asarray(b_out, dtype=np.float32)
    try:
        out, _ = run(x, w_qkv, w_out, b_out, trace=False)
    except Exception:
        # one retry for transient device errors
        out, _ = run(x, w_qkv, w_out, b_out, trace=False)
    return out



# revision 7
# speedup vs baseline: 1.1972x; 1.1972x over previous
"""Trainium2 Bass kernel for multi-head self-attention (dense transformer block).

Problem: x[4, 2048, 1024], w_qkv[3072, 1024], w_out[1024, 1024], b_out[1024]
  qkv = x @ w_qkv.T, rearranged 'b t (d k h) -> k b h t d' (k=3, h=16)
  attn = softmax(q @ k.T * DIM**-0.5); out = (attn @ v) concat heads @ w_out.T + b_out

Sharding (8 cores): data-parallel over batch b (4) x tensor-parallel over
head-groups (2 groups of 8 heads).  Each core gets x[b] (bf16, transposed on
host), its head-group's qkv weight columns (bf16; q/k columns PERMUTED so the
projection directly emits the fp8 DoubleRow operand layout), and the matching
w_out columns; it produces a partial [T, DIM] fp32 output which the host sums
per batch pair and adds b_out.

Device-side dataflow per core (8 heads, T=2048, DH=64):
  - K/Q projections (bf16) write PSUM column-blocks (g2, kt) whose partition
    is h4*32+dlo; DVE fp32->fp8e4 copies land them in q8/k8
    [32*h4+dlo, g2, kt, t] - the [32, 2, *] layout MatmulPerfMode.DoubleRow
    wants, so scores run at 0.5 cycles/row (2x the bf16 contraction-64 path).
  - scores S^T[j, i] per head: 16 j-blocks in groups [6, 6, 4] per 3-bank
    PSUM tile; ScalarE exp (softmax scale folded in) -> bf16 e tiles.
  - AV: e.T @ [v | 1] per (head, 128-query block) in bf16; the ones column
    gives the softmax denominator; DVE reciprocal+scale -> attn bf16.
  - attn -> attnT via DMA-transpose (XBAR); bf16 out-proj against woT;
    fp32 partial DMA'd out.

Schedule: phase A streams K(g2=0)+Q0 (~17us) so ScalarE's exp stream starts
early; K(g2=1) rides units 0-3 and V units 4-11 (re-streamed x); AVs lag
behind a 13-deep e-tile ring (which recycles the retired wk/wv buffers);
transposes/projection trail one chunk behind.
"""

import math
from contextlib import ExitStack
from dataclasses import dataclass

import numpy as np

import concourse.bass as bass
import concourse.mybir as mybir
import concourse.tile as tile
from concourse import bacc
from concourse.bass_utils import run_bass_kernel_spmd

F32 = mybir.dt.float32
BF16 = mybir.dt.bfloat16
FP8 = mybir.dt.float8e4
P = 128
DR = mybir.MatmulPerfMode.DoubleRow


@dataclass(frozen=True)
class Cfg:
    T: int = 2048      # sequence length
    DIM: int = 1024    # model dim (= qkv contraction dim)
    NH: int = 8        # heads per core
    DH: int = 64       # head dim
    SCALE: float = 1024.0 ** -0.5

    @property
    def CB(self):      # contraction blocks of 128 over DIM
        return self.DIM // P

    @property
    def OD(self):      # per-core attention width = NH*DH
        return self.NH * self.DH

    @property
    def OB(self):      # o-blocks of 128 over OD
        return self.OD // P

    @property
    def JB(self):      # key blocks of 128
        return self.T // P

    @property
    def ICSZ(self):    # query chunk size
        return 256

    @property
    def NIC(self):     # number of query chunks
        return self.T // self.ICSZ

    @property
    def IB(self):      # query blocks of 128 per chunk
        return self.ICSZ // P

    @property
    def TCH(self):     # t-chunk for projection rhs streaming
        return 256

    @property
    def NTCH(self):
        return self.T // self.TCH


def _emit_kernel(tc, cfg, xT, wq, wk, wv, woT, out):
    nc = tc.nc
    c = cfg
    VW = c.DH + 1          # per-head V width incl. ones column
    JGROUPS = [6, 6, 4]    # j-blocks per scores PSUM tile / exp instruction

    ctx = ExitStack()
    with ctx:
        persist = ctx.enter_context(tc.tile_pool(name="persist", bufs=1))
        mmp = ctx.enter_context(tc.tile_pool(name="mmp", bufs=2, space="PSUM"))
        smp = ctx.enter_context(tc.tile_pool(name="smp", bufs=2, space="PSUM"))

        # fp8 q/k in DoubleRow layout: [32*h4+dlo, g2, kt, t]; h=g2*4+h4,
        # d=kt*32+dlo
        q8 = persist.tile([P, 2, 2, c.T], FP8, name="q8", tag="q8")
        k8 = persist.tile([P, 2, 2, c.T], FP8, name="k8", tag="k8")
        v_sb = persist.tile([P, c.JB, c.NH, VW], BF16, name="v_sb", tag="v")
        woT_sb = persist.tile([P, c.OB, c.DIM], BF16, name="woT_sb", tag="woT")
        wq_sb = persist.tile([P, c.CB, c.OD], BF16, name="wq_sb", tag="wq")

        nc.gpsimd.memset(v_sb[:, :, :, c.DH : c.DH + 1], 1.0)

        xT_r = xT.rearrange("(cb p) t -> p cb t", p=P)
        wq_r = wq.rearrange("(cb p) o -> p cb o", p=P)
        wk_r = wk.rearrange("(cb p) o -> p cb o", p=P)
        wv_r = wv.rearrange("(cb p) o -> p cb o", p=P)

        wkvp = ctx.enter_context(tc.tile_pool(name="wkvp", bufs=1))
        ep = ctx.enter_context(tc.tile_pool(name="ep", bufs=12))
        xp = ctx.enter_context(tc.tile_pool(name="xp", bufs=3))
        xk2 = ctx.enter_context(tc.tile_pool(name="xk2", bufs=3))
        xq = ctx.enter_context(tc.tile_pool(name="xq", bufs=2))
        ap = ctx.enter_context(tc.tile_pool(name="ap", bufs=2))
        atp = ctx.enter_context(tc.tile_pool(name="atp", bufs=2))
        op = ctx.enter_context(tc.tile_pool(name="op", bufs=2))
        rp = ctx.enter_context(tc.tile_pool(name="rp", bufs=4))

        wk_sb = wkvp.tile([P, c.CB, c.OD], BF16, name="wk_sb", tag="wk")
        wv_sb = wkvp.tile([P, c.CB, c.OD], BF16, name="wv_sb", tag="wv")
        nc.sync.dma_start(out=wk_sb, in_=wk_r)
        nc.sync.dma_start(out=wq_sb, in_=wq_r)
        nc.sync.dma_start(out=wv_sb, in_=wv_r)
        nc.sync.dma_start(
            out=woT_sb, in_=woT.rearrange("(ob p) n -> p ob n", p=P)
        )

        def kq_piece(dst8, w_sb, blk, x_t, tdst):
            """One (g2, kt) column-block x one t-chunk of the K/Q projection,
            landed as fp8 DoubleRow layout."""
            g2, kt = blk // 2, blk % 2
            ps = smp.tile([P, c.TCH], F32, name="ps_kq", tag="sm")
            for cb in range(c.CB):
                nc.tensor.matmul(
                    ps,
                    w_sb[:, cb, bass.ts(blk, P)],
                    x_t[:, cb, :],
                    start=(cb == 0),
                    stop=(cb == c.CB - 1),
                )
            nc.vector.tensor_copy(
                out=dst8[:, g2, kt, bass.ts(tdst, c.TCH)], in_=ps
            )

        def v_piece(x_t, tch):
            """V projection for one t-chunk (2 t-blocks of 128)."""
            for tbl in range(c.TCH // P):
                ps_v = smp.tile([P, c.OD], F32, name="ps_v", tag="sm")
                for cb in range(c.CB):
                    nc.tensor.matmul(
                        ps_v,
                        x_t[:, cb, bass.ts(tbl, P)],
                        wv_sb[:, cb, :],
                        start=(cb == 0),
                        stop=(cb == c.CB - 1),
                    )
                tb = tch * (c.TCH // P) + tbl
                nc.vector.tensor_copy(
                    out=v_sb[:, tb, :, 0 : c.DH],
                    in_=ps_v.rearrange("p (h d) -> p h d", h=c.NH),
                )

        def emit_scores(ic, h):
            """DoubleRow fp8 scores + ScalarE exp for head h, chunk ic."""
            g2, h4 = h // 4, h % 4
            rows = slice(32 * h4, 32 * (h4 + 1))
            e = ep.tile([P, c.JB, c.ICSZ], BF16, name=f"e_{ic}_{h}", tag="e")
            jb0 = 0
            for jj in JGROUPS:
                ps = mmp.tile([P, 6, c.ICSZ], F32, name="ps_s", tag="mm")
                for j in range(jj):
                    nc.tensor.matmul(
                        ps[:, j, :],
                        k8[rows, g2, :, bass.ts(jb0 + j, P)],
                        q8[rows, g2, :, bass.ts(ic, c.ICSZ)],
                        start=True,
                        stop=True,
                        perf_mode=DR,
                        tile_position=(32 * h4, 0),
                    )
                nc.scalar.activation(
                    out=e[:, jb0 : jb0 + jj, :],
                    in_=ps[:, 0:jj, :],
                    func=mybir.ActivationFunctionType.Exp,
                    scale=c.SCALE,
                )
                jb0 += jj
            return e

        attn_tiles = {}
        attnT_tiles = {}

        def emit_av(ic, h, e):
            """attn[i, dh] = norm(e.T @ [v|1]) for head h."""
            if ic not in attn_tiles:
                attn_tiles[ic] = ap.tile(
                    [P, c.IB, c.NH, c.DH], BF16, name=f"attn_{ic}", tag="attn"
                )
            attn_sb = attn_tiles[ic]
            for ib in range(c.IB):
                ps_av = smp.tile([P, c.OD], F32, name="ps_av", tag="sm")
                for jb in range(c.JB):
                    nc.tensor.matmul(
                        ps_av[:, 0:VW],
                        e[:, jb, bass.ts(ib, P)],
                        v_sb[:, jb, h, :],
                        start=(jb == 0),
                        stop=(jb == c.JB - 1),
                    )
                rec = rp.tile([P, 1], F32, name="rec", tag="rec")
                nc.vector.reciprocal(rec, ps_av[:, c.DH : c.DH + 1])
                nc.vector.tensor_scalar_mul(
                    out=attn_sb[:, ib, h, :],
                    in0=ps_av[:, 0 : c.DH],
                    scalar1=rec,
                )

        def emit_transpose(ic):
            """DMA-transpose (XBAR) attn chunk ic into [od, i] layout."""
            attnT_tiles[ic] = atp.tile(
                [P, c.OB, c.ICSZ], BF16, name=f"attnT_{ic}", tag="attnT"
            )
            attnT_sb = attnT_tiles[ic]
            attn_sb = attn_tiles.pop(ic)
            for ib in range(c.IB):
                for hp in range(c.OB):
                    nc.sync.dma_start_transpose(
                        out=attnT_sb[:, hp, bass.ts(ib, P)],
                        in_=attn_sb[:, ib, 2 * hp : 2 * hp + 2, :],
                    )

        def emit_proj_piece(ic, tb):
            """Project one 128-query block of a finished chunk + store."""
            attnT_sb = attnT_tiles[ic]
            t0 = ic * c.ICSZ + tb * P
            for occ in range(2):
                ps_o = smp.tile([P, c.DIM // 2], F32, name="ps_o", tag="sm")
                for ob in range(c.OB):
                    nc.tensor.matmul(
                        ps_o,
                        attnT_sb[:, ob, bass.ts(tb, P)],
                        woT_sb[:, ob, bass.ts(occ, c.DIM // 2)],
                        start=(ob == 0),
                        stop=(ob == c.OB - 1),
                    )
                o_sb = op.tile([P, c.DIM // 2], F32, name="o_sb", tag="ost")
                nc.vector.tensor_copy(out=o_sb, in_=ps_o)
                nc.sync.dma_start(
                    out=out[t0 : t0 + P, bass.ts(occ, c.DIM // 2)], in_=o_sb
                )
            if tb == c.IB - 1:
                attnT_tiles.pop(ic)

        # ---------------- phase A: K(g2=0) + Q0 (all blocks) ----------------
        for tch in range(c.NTCH):
            x_t = xp.tile([P, c.CB, c.TCH], BF16, name=f"x_{tch}", tag="x")
            nc.sync.dma_start(out=x_t, in_=xT_r[:, :, bass.ts(tch, c.TCH)])
            kq_piece(k8, wk_sb, 0, x_t, tch)
            kq_piece(k8, wk_sb, 1, x_t, tch)
            if tch == 0:
                for blk in range(4):
                    kq_piece(q8, wq_sb, blk, x_t, 0)

        # prefetch x chunks for the K(g2=1) pass riding units 0-3
        xk_tiles = {}

        def xk_dma(tch):
            x_t = xk2.tile([P, c.CB, c.TCH], BF16, name=f"xk_{tch}", tag="xk")
            nc.sync.dma_start(out=x_t, in_=xT_r[:, :, bass.ts(tch, c.TCH)])
            xk_tiles[tch] = x_t

        xk_dma(0)
        xk_dma(1)

        # ---------------- main loop: 64 units of (ic, h) ----------------
        av_queue = []      # (ic, h, e) awaiting AV emission (gated on V)
        late_q = []        # PE work that trails: proj pieces
        v_done_unit = 10   # V pieces ride units 4..10
        x2_cur = None

        units = [(ic, h) for ic in range(c.NIC) for h in range(c.NH)]
        for u, (ic, h) in enumerate(units):
            e = emit_scores(ic, h)

            if u < 4:
                # K(g2=1): two t-chunks per unit
                for tch in (2 * u, 2 * u + 1):
                    kq_piece(k8, wk_sb, 2, xk_tiles[tch], tch)
                    kq_piece(k8, wk_sb, 3, xk_tiles[tch], tch)
                    if tch + 2 < c.NTCH:
                        xk_dma(tch + 2)
                    else:
                        # re-stream x for the V pass (same ring)
                        xk_dma(tch + 2 - c.NTCH)
            elif u <= v_done_unit:
                # V: 8 t-chunks over units 4..v_done_unit
                nv = v_done_unit - 3
                lo = (u - 4) * c.NTCH // nv
                hi = (u - 3) * c.NTCH // nv
                for tch in range(lo, hi):
                    v_piece(xk_tiles[tch], tch)
                    if tch + 2 < c.NTCH:
                        xk_dma(tch + 2)

            # JIT q for chunk ic+1: x2 DMA at h==0, blocks at odd h
            if ic + 1 < c.NIC:
                if h == 0:
                    x2_cur = xq.tile([P, c.CB, c.TCH], BF16, name="x2", tag="xq")
                    nc.sync.dma_start(
                        out=x2_cur, in_=xT_r[:, :, bass.ts(ic + 1, c.TCH)]
                    )
                if h % 2 == 1:
                    kq_piece(q8, wq_sb, h // 2, x2_cur, ic + 1)

            av_queue.append((ic, h, e))

            if u > v_done_unit:
                # drain up to 2 AVs + 1 trailing proj piece per unit
                for _ in range(2):
                    if av_queue:
                        aic, ah, ae = av_queue.pop(0)
                        emit_av(aic, ah, ae)
                        if ah == c.NH - 1:
                            emit_transpose(aic)
                            late_q.extend(
                                (aic, tb) for tb in range(c.IB)
                            )
                if late_q:
                    pic, tb = late_q.pop(0)
                    emit_proj_piece(pic, tb)

        # ---------------- drain ----------------
        while av_queue:
            aic, ah, ae = av_queue.pop(0)
            emit_av(aic, ah, ae)
            if ah == c.NH - 1:
                emit_transpose(aic)
                late_q.extend((aic, tb) for tb in range(c.IB))
        while late_q:
            pic, tb = late_q.pop(0)
            emit_proj_piece(pic, tb)


def build_nc(cfg: Cfg = Cfg(), reps: int = 1):
    nc = bacc.Bacc()
    xT = nc.declare_dram_parameter("xT", [cfg.DIM, cfg.T], BF16, isOutput=False)
    wq = nc.declare_dram_parameter("wq", [cfg.DIM, cfg.OD], BF16, isOutput=False)
    wk = nc.declare_dram_parameter("wk", [cfg.DIM, cfg.OD], BF16, isOutput=False)
    wv = nc.declare_dram_parameter("wv", [cfg.DIM, cfg.OD], BF16, isOutput=False)
    woT = nc.declare_dram_parameter("woT", [cfg.OD, cfg.DIM], BF16, isOutput=False)
    out = nc.declare_dram_parameter("out", [cfg.T, cfg.DIM], F32, isOutput=True)
    with tile.TileContext(nc) as tc:
        for _ in range(reps):
            _emit_kernel(tc, cfg, xT[:], wq[:], wk[:], wv[:], woT[:], out[:])
    nc.finalize()
    return nc


def prepare_core_inputs(x, w_qkv, w_out, b, g, cfg: Cfg, n_groups: int):
    """Host-side shard prep for core (batch b, head-group g)."""
    import ml_dtypes

    H = cfg.NH * n_groups
    heads = np.arange(cfg.NH * g, cfg.NH * (g + 1))
    bf16 = ml_dtypes.bfloat16

    # w_qkv row for (k, head h, dim d) is d*(3*H) + k*H + h
    def gather_perm(k_idx):
        # DoubleRow-permuted columns: c = g2*256 + kt*128 + h4*32 + dlo,
        # head h = heads[g2*4 + h4], d = kt*32 + dlo
        cols = np.empty(cfg.OD, dtype=np.int64)
        for g2 in range(2):
            for kt in range(2):
                for h4 in range(4):
                    h = heads[g2 * 4 + h4]
                    d = kt * 32 + np.arange(32)
                    c0 = g2 * 256 + kt * 128 + h4 * 32
                    cols[c0 : c0 + 32] = d * (3 * H) + k_idx * H + h
        return np.ascontiguousarray(w_qkv[cols, :].T).astype(bf16)

    def gather_std(k_idx):
        d = np.arange(cfg.DH)
        rows = (d[None, :] * (3 * H) + k_idx * H + heads[:, None]).reshape(-1)
        return np.ascontiguousarray(w_qkv[rows, :].T).astype(bf16)

    return {
        "xT": np.ascontiguousarray(x[b].T).astype(bf16),
        "wq": gather_perm(0),
        "wk": gather_perm(1),
        "wv": gather_std(2),
        "woT": np.ascontiguousarray(
            w_out[:, cfg.OD * g : cfg.OD * (g + 1)].T
        ).astype(bf16),
    }


_NC_CACHE = {}


def _get_nc(cfg: Cfg):
    if cfg not in _NC_CACHE:
        _NC_CACHE[cfg] = build_nc(cfg)
    return _NC_CACHE[cfg]


def run(x, w_qkv, w_out, b_out, trace=False):
    """Shard, execute on 8 cores, gather. Returns (out, BassKernelResults)."""
    cfg = Cfg()
    B, T, DIM = x.shape
    assert (T, DIM) == (cfg.T, cfg.DIM), (x.shape, cfg)
    n_groups = 2
    nc = _get_nc(cfg)
    in_maps = [
        prepare_core_inputs(x, w_qkv, w_out, b, g, cfg, n_groups)
        for b in range(B)
        for g in range(n_groups)
    ]
    res = run_bass_kernel_spmd(
        nc, in_maps, core_ids=list(range(len(in_maps))), trace=trace
    )
    out = np.empty((B, T, DIM), dtype=np.float32)
    for b in range(B):
        out[b] = res.results[2 * b]["out"] + res.results[2 * b + 1]["out"]
    out += b_out.astype(np.float32)
    return out, res


def _make_pjrt_fn(nc, in_maps):
    """Build a non-donating jitted 8-core runner for a prebuilt nc."""
    import jax
    import numpy as np_
    from jax.sharding import Mesh, PartitionSpec
    from jax.experimental.shard_map import shard_map

    from concourse import bass2jax

    bass2jax.install_neuronx_cc_hook()
    n_cores = len(in_maps)
    partition_name = nc.partition_id_tensor.name if nc.partition_id_tensor else None
    in_names, out_names, out_avals, zero_outs = [], [], [], []
    for alloc in nc.m.functions[0].allocations:
        if not isinstance(alloc, mybir.MemoryLocationSet):
            continue
        name = alloc.memorylocations[0].name
        if alloc.kind == "ExternalInput":
            if name != partition_name:
                in_names.append(name)
        elif alloc.kind == "ExternalOutput":
            shape = tuple(alloc.tensor_shape)
            dtype = mybir.dt.np(alloc.dtype)
            out_names.append(name)
            out_avals.append(jax.core.ShapedArray(shape, dtype))
            zero_outs.append(np_.zeros(shape, dtype))
    n_params = len(in_names)
    all_in_names = in_names + out_names
    if partition_name is not None:
        all_in_names = all_in_names + [partition_name]

    def _body(*args):
        operands = list(args)
        if partition_name is not None:
            operands.append(bass2jax.partition_id_tensor())
        return tuple(
            bass2jax._bass_exec_p.bind(
                *operands,
                out_avals=tuple(out_avals),
                in_names=tuple(all_in_names),
                out_names=tuple(out_names),
                lowering_input_output_aliases=(),
                sim_require_finite=True,
                sim_require_nnan=True,
                nc=nc,
            )
        )

    devices = jax.devices()[:n_cores]
    mesh = Mesh(np_.asarray(devices), ("core",))
    nin = n_params + len(out_names)
    f = jax.jit(
        shard_map(
            _body,
            mesh=mesh,
            in_specs=(PartitionSpec("core"),) * nin,
            out_specs=(PartitionSpec("core"),) * len(out_names),
            check_rep=False,
        ),
        keep_unused=True,
    )
    concat_in = [
        np_.concatenate([np_.asarray(in_maps[c][n]) for c in range(n_cores)], axis=0)
        for n in in_names
    ] + [np_.zeros((n_cores * z.shape[0], *z.shape[1:]), z.dtype) for z in zero_outs]
    dev_in = jax.device_put(concat_in)
    return f, dev_in


def _time_fn(f, dev_in, calls=4, rounds=6):
    import time

    import jax

    r = f(*dev_in)
    jax.block_until_ready(r)
    best = float("inf")
    for _ in range(rounds):
        t0 = time.perf_counter()
        rs = [f(*dev_in) for _ in range(calls)]
        jax.block_until_ready(rs)
        best = min(best, (time.perf_counter() - t0) / calls)
    return best


def time_hw(x, w_qkv, w_out, b_out, reps=(4, 36)):
    """Marginal-cost HW timing: per-call time of an R2-repeat NEFF minus an
    R1-repeat NEFF, over (R2-R1), cancels the axon dispatch overhead."""
    cfg = Cfg()
    B = x.shape[0]
    in_maps = [
        prepare_core_inputs(x, w_qkv, w_out, b, g, cfg, 2)
        for b in range(B)
        for g in range(2)
    ]
    r1, r2 = reps
    ncA = build_nc(cfg, reps=r1)
    fA, devA = _make_pjrt_fn(ncA, in_maps)
    tA = _time_fn(fA, devA)
    ncB = build_nc(cfg, reps=r2)
    fB, devB = _make_pjrt_fn(ncB, in_maps)
    tB = _time_fn(fB, devB)
    per_exec = (tB - tA) / (r2 - r1)
    return tA, per_exec


def kernel(x, w_qkv, w_out, b_out):
    x = np.asarray(x, dtype=np.float32)
    w_qkv = np.asarray(w_qkv, dtype=np.float32)
    w_out = np.asarray(w_out, dtype=np.float32)
    b_out = np.asarray(b_out, dtype=np.float32)
    try:
        out, _ = run(x, w_qkv, w_out, b_out, trace=False)
    except Exception:
        # one retry for transient device errors
        out, _ = run(x, w_qkv, w_out, b_out, trace=False)
    return out
